# revision 1
# baseline (speedup 1.0000x reference)
"""Trainium2 Bass kernel for 3x3 SAME conv (NHWC, 16x512x512x16, C=16) + bias.

Strategy (8 NeuronCores, data-parallel over batch; 2 images per core):
  - Host casts x to bf16 and flattens per-core images into one padded buffer.
  - im2col tiles are materialized by XBAR transpose-DMA straight from DRAM:
    group of G=8 output pixels needs a 160-element window (10 w-positions x
    16 channels); windows at stride 128 tile the row seamlessly, so one 2D
    transpose-DMA loads a whole chunk:
       ICa[128, cols] : window elems [128c-16, 128c+112)   (wi 0..7)
       ICc[32,  cols] : window elems [128c+112, 128c+144)  (wi 8..9)
  - Conv = 6 accumulating bf16 matmuls per PSUM bank (3 filter rows x 2
    K-parts) with host-prebuilt banded weight matrices:
       lhsT Wa[dy] [128,128], Wc[dy] [32,128]; psum [128, 8 rows x 64 groups]
  - DVE adds bias (fp32) while copying PSUM->SBUF; one store DMA per chunk
    writes a [row, (j,co), n] device layout; host reorders to NHWC.
"""

from contextlib import ExitStack

import ml_dtypes
import numpy as np

import concourse.bass as bass
import concourse.bacc as bacc
import concourse.mybir as mybir
import concourse.tile as tile
from concourse.bass_utils import run_bass_kernel_spmd

F32 = mybir.dt.float32
BF16 = mybir.dt.bfloat16

N_CORES = 8
H = 512
W = 512
C = 16
IMG = 2                  # images per core
G = 8                    # output pixels per group
NGR = W // G             # 64 groups per row
RB = 8                   # output rows per PSUM bank
NBANK = 4                # banks per chunk
RC = RB * NBANK          # 32 output rows per chunk
NCHUNK = H // RC         # 16 chunks per image
TIN = RC + 2             # 34 input-row tiles per chunk (halo)
ROW = W * C              # 8192 elements per image row
NROWS = IMG * H          # 1024 rows per core
FRONT = 128              # front shim (covers the -16 window offset)
XLEN = (NROWS + 2) * ROW + FRONT + 256


def _build_nc():
    nc = bacc.Bacc(None, target_bir_lowering=False)
    x = nc.dram_tensor("x", [XLEN], BF16, kind="ExternalInput")
    wa = nc.dram_tensor("wa", [128, 3, 128], BF16, kind="ExternalInput")
    wc = nc.dram_tensor("wc", [32, 3, 128], BF16, kind="ExternalInput")
    bias = nc.dram_tensor("bias", [128, 1], F32, kind="ExternalInput")
    zeros = nc.dram_tensor("zeros", [1024], BF16, kind="ExternalInput")
    out = nc.dram_tensor("out", [IMG, H, 128, NGR], F32, kind="ExternalOutput")

    with ExitStack() as ctx:
        tc = ctx.enter_context(tile.TileContext(nc))
        wpool = ctx.enter_context(tc.tile_pool(name="w", bufs=1))
        icpool = ctx.enter_context(tc.tile_pool(name="ic", bufs=3))
        opool = ctx.enter_context(tc.tile_pool(name="o", bufs=3))
        pspool = ctx.enter_context(tc.tile_pool(name="ps", bufs=8, space="PSUM"))

        wat = wpool.tile([128, 3, 128], BF16)
        nc.sync.dma_start(wat[:, :, :], wa[:, :, :])
        wct = wpool.tile([32, 3, 128], BF16)
        nc.sync.dma_start(wct[:, :, :], wc[:, :, :])
        bias_t = wpool.tile([128, 1], F32)
        nc.sync.dma_start(bias_t[:, :], bias[:, :])

        for img in range(IMG):
            for ck in range(NCHUNK):
                r0 = ck * RC
                R0 = img * H + r0          # global row index
                off = FRONT + R0 * ROW - 16

                ICa = icpool.tile([128, TIN, NGR], BF16, tag="ica")
                ICc = icpool.tile([32, TIN, NGR], BF16, tag="icc")
                nc.sync.dma_start_transpose(
                    ICa[:, :, :].rearrange("p t n -> p (t n)"),
                    bass.AP(x, off, [[128, TIN * NGR], [1, 128]]),
                )
                nc.sync.dma_start_transpose(
                    ICc[:, :, :].rearrange("p t n -> p (t n)"),
                    bass.AP(x, off + 128, [[128, TIN * NGR], [1, 32]]),
                )
                # SAME-pad zero patches: left (w'=-1) / right (w'=512)
                nc.gpsimd.memset(ICa[0:16, :, 0], 0.0)
                nc.sync.dma_start(
                    ICc[16:32, :, NGR - 1],
                    bass.AP(zeros, 0, [[TIN, 16], [1, TIN]]),
                )
                # image-boundary pad rows
                if ck == 0:
                    nc.gpsimd.memset(ICa[:, 0, :], 0.0)
                    nc.gpsimd.memset(ICc[:, 0, :], 0.0)
                if ck == NCHUNK - 1:
                    nc.gpsimd.memset(ICa[:, TIN - 1, :], 0.0)
                    nc.gpsimd.memset(ICc[:, TIN - 1, :], 0.0)

                O = opool.tile([128, RC, NGR], F32, tag="o")
                for b in range(NBANK):
                    ps = pspool.tile([128, RB, NGR], F32, tag="ps")
                    for dy in range(3):
                        t0 = RB * b + dy
                        nc.tensor.matmul(
                            ps[:, :, :], wat[:, dy, :], ICa[:, t0:t0 + RB, :],
                            start=(dy == 0), stop=False)
                        nc.tensor.matmul(
                            ps[:, :, :], wct[:, dy, :], ICc[:, t0:t0 + RB, :],
                            start=False, stop=(dy == 2))
                    nc.vector.tensor_scalar_add(
                        out=O[:, RB * b:RB * (b + 1), :], in0=ps[:, :, :],
                        scalar1=bias_t[:, 0:1])

                nc.sync.dma_start(
                    bass.AP(out, (img * H + r0) * ROW,
                            [[NGR, 128], [ROW, RC], [1, NGR]]),
                    O[:, :, :],
                )
    nc.finalize()
    return nc


_NC_CACHE = None


def _get_nc():
    global _NC_CACHE
    if _NC_CACHE is None:
        _NC_CACHE = _build_nc()
    return _NC_CACHE


def _banded_weights(filters: np.ndarray):
    """filters (3,3,16,16) HWIO -> wa [128,3,128], wc [32,3,128] bf16 banded."""
    wb = np.zeros((3, 160, 128), np.float32)
    for dy in range(3):
        for j in range(G):
            for d in range(3):
                wi = j + d
                wb[dy, wi * 16:(wi + 1) * 16, j * 16:(j + 1) * 16] = filters[dy, d]
    wa = np.ascontiguousarray(wb[:, :128, :].transpose(1, 0, 2)).astype(
        ml_dtypes.bfloat16)
    wc = np.ascontiguousarray(wb[:, 128:, :].transpose(1, 0, 2)).astype(
        ml_dtypes.bfloat16)
    return wa, wc


def _prep_inputs(x, filters, bias):
    x = np.asarray(x, dtype=np.float32)
    filters = np.asarray(filters, dtype=np.float32)
    bias = np.asarray(bias, dtype=np.float32)
    assert x.shape == (16, H, W, C), x.shape

    wa, wc = _banded_weights(filters)
    bias128 = np.ascontiguousarray(
        np.tile(bias, G).reshape(128, 1)).astype(np.float32)
    zeros = np.zeros(1024, ml_dtypes.bfloat16)

    x_bf = x.astype(ml_dtypes.bfloat16)
    in_maps = []
    for i in range(N_CORES):
        xd = np.zeros(XLEN, ml_dtypes.bfloat16)
        xd[FRONT + ROW:FRONT + ROW + NROWS * ROW] = \
            x_bf[i * IMG:(i + 1) * IMG].reshape(-1)
        in_maps.append(
            {"x": xd, "wa": wa, "wc": wc, "bias": bias128, "zeros": zeros})
    return in_maps


def _assemble(results) -> np.ndarray:
    dev = np.concatenate([r["out"] for r in results], axis=0)
    # dev [16, 512, (j,co)=128, n=64] -> NHWC [16, 512, w=8n+j, co]
    out = dev.reshape(16, H, G, C, NGR).transpose(0, 1, 4, 2, 3)
    return np.ascontiguousarray(out.reshape(16, H, W, C))


def kernel(x: np.ndarray, filters: np.ndarray, bias: np.ndarray) -> np.ndarray:
    in_maps = _prep_inputs(x, filters, bias)
    nc = _get_nc()
    res = run_bass_kernel_spmd(nc, in_maps, core_ids=list(range(N_CORES)))
    return _assemble(res.results)



# revision 2
# speedup vs baseline: 8.6097x; 8.6097x over previous
"""Trainium2 Bass kernel for 3x3 SAME conv (NHWC, 16x512x512x16, C=16) + bias.

Strategy (8 NeuronCores, data-parallel over batch; 2 images per core):
  - Host pre-arranges x (bf16) into a partition-blocked layout
      xd[(wi,ci)=128][img, rpad, g=0..64]
    where group g covers w = 8g-1+wi (SAME pad baked in as zero cols/rows).
    Every device DMA is then plain and per-partition contiguous - no
    transpose DMAs, no sub-512B packets.
  - Conv per 8-row PSUM bank = 6 accumulating bf16 matmuls:
      3x center (K=128): Wa[dy]^T @ X[:, t+dy : t+dy+8, 0:64]
      3x tail   (K=32):  Wt[dy]^T @ X[0:32, t+dy : t+dy+8, 1:65]
    (the im2col "tail" wi=8,9 lives at group g+1, partitions 0..31 of the
    SAME tile - zero duplication.)
  - DVE adds bias during PSUM->SBUF copy, casting to bf16; one store DMA
    per chunk writes partition-blocked contiguous runs; host reorders to
    NHWC f32.
"""

from contextlib import ExitStack

import ml_dtypes
import numpy as np

import concourse.bass as bass
import concourse.bacc as bacc
import concourse.mybir as mybir
import concourse.tile as tile
from concourse.bass_utils import run_bass_kernel_spmd

F32 = mybir.dt.float32
BF16 = mybir.dt.bfloat16

N_CORES = 8
H = 512
W = 512
C = 16
IMG = 2                  # images per core
G = 8                    # output pixels per group
NG = 65                  # padded groups per row (g=0..64; g holds w=8g-1+wi)
NGO = 64                 # output groups per row
RC = 64                  # output rows per chunk
NCHUNK = H // RC         # 8 chunks per image
TIN = RC + 2             # input rows per chunk (halo)
RB = 8                   # output rows per PSUM bank
NBANK = RC // RB         # 8 banks per chunk
HP = H + 2               # padded rows per image
PBLK = IMG * HP * NG     # per-partition elements in xd
OBLK = IMG * H * NGO     # per-partition elements in out


def _build_nc():
    nc = bacc.Bacc(None, target_bir_lowering=False)
    xd = nc.dram_tensor("xd", [128 * PBLK], BF16, kind="ExternalInput")
    wa = nc.dram_tensor("wa", [128, 3, 128], BF16, kind="ExternalInput")
    wt = nc.dram_tensor("wt", [32, 3, 128], BF16, kind="ExternalInput")
    bias = nc.dram_tensor("bias", [128, 1], F32, kind="ExternalInput")
    out = nc.dram_tensor("out", [128 * OBLK], BF16, kind="ExternalOutput")

    with ExitStack() as ctx:
        tc = ctx.enter_context(tile.TileContext(nc))
        wpool = ctx.enter_context(tc.tile_pool(name="w", bufs=1))
        xpool = ctx.enter_context(tc.tile_pool(name="x", bufs=3))
        opool = ctx.enter_context(tc.tile_pool(name="o", bufs=2))
        pspool = ctx.enter_context(tc.tile_pool(name="ps", bufs=8, space="PSUM"))

        wat = wpool.tile([128, 3, 128], BF16)
        nc.sync.dma_start(wat[:, :, :], wa[:, :, :])
        wtt = wpool.tile([32, 3, 128], BF16)
        nc.sync.dma_start(wtt[:, :, :], wt[:, :, :])
        bias_t = wpool.tile([128, 1], F32)
        nc.sync.dma_start(bias_t[:, :], bias[:, :])

        for img in range(IMG):
            for ck in range(NCHUNK):
                r0 = ck * RC

                X = xpool.tile([128, TIN, NG], BF16, tag="x")
                nc.sync.dma_start(
                    X[:, :, :].rearrange("p t g -> p (t g)"),
                    bass.AP(xd, (img * HP + r0) * NG,
                            [[PBLK, 128], [1, TIN * NG]]),
                )

                O = opool.tile([128, RC, NGO], BF16, tag="o")
                for b in range(NBANK):
                    t0 = RB * b
                    ps = pspool.tile([128, RB, NGO], F32, tag="ps")
                    for dy in range(3):
                        nc.tensor.matmul(
                            ps[:, :, :], wat[:, dy, :],
                            X[:, t0 + dy:t0 + dy + RB, 0:NGO],
                            start=(dy == 0), stop=False)
                    for dy in range(3):
                        nc.tensor.matmul(
                            ps[:, :, :], wtt[:, dy, :],
                            X[0:32, t0 + dy:t0 + dy + RB, 1:NG],
                            start=False, stop=(dy == 2))
                    nc.vector.tensor_scalar_add(
                        out=O[:, t0:t0 + RB, :], in0=ps[:, :, :],
                        scalar1=bias_t[:, 0:1])

                nc.sync.dma_start(
                    bass.AP(out, (img * H + r0) * NGO,
                            [[OBLK, 128], [1, RC * NGO]]),
                    O[:, :, :].rearrange("p t g -> p (t g)"),
                )
    nc.finalize()
    return nc


_NC_CACHE = None


def _get_nc():
    global _NC_CACHE
    if _NC_CACHE is None:
        _NC_CACHE = _build_nc()
    return _NC_CACHE


def _weights(filters: np.ndarray):
    """filters (3,3,16,16) HWIO -> wa [128,3,128], wt [32,3,128] bf16."""
    wa = np.zeros((3, 8, 16, 8, 16), np.float32)   # [dy, wi, ci, j, co]
    wt = np.zeros((3, 2, 16, 8, 16), np.float32)   # [dy, k, ci, j, co]
    for dy in range(3):
        for dx in range(3):
            for j in range(G):
                wi = j + dx
                if wi <= 7:
                    wa[dy, wi, :, j, :] = filters[dy, dx]
                else:
                    wt[dy, wi - 8, :, j, :] = filters[dy, dx]
    wa_d = np.ascontiguousarray(
        wa.reshape(3, 128, 128).transpose(1, 0, 2)).astype(ml_dtypes.bfloat16)
    wt_d = np.ascontiguousarray(
        wt.reshape(3, 32, 128).transpose(1, 0, 2)).astype(ml_dtypes.bfloat16)
    return wa_d, wt_d


def _prep_inputs(x, filters, bias):
    x = np.asarray(x, dtype=np.float32)
    filters = np.asarray(filters, dtype=np.float32)
    bias = np.asarray(bias, dtype=np.float32)
    assert x.shape == (16, H, W, C), x.shape

    wa_d, wt_d = _weights(filters)
    bias128 = np.ascontiguousarray(
        np.tile(bias, G).reshape(128, 1)).astype(np.float32)

    # x -> xd[(wi,ci), img, rpad, g]: w-padded to idx 0..519 (w = idx-1)
    x_bf = x.astype(ml_dtypes.bfloat16)
    in_maps = []
    for i in range(N_CORES):
        xc = x_bf[i * IMG:(i + 1) * IMG]                  # [2, 512, 512, 16]
        xp = np.zeros((IMG, H, 520, C), ml_dtypes.bfloat16)
        xp[:, :, 1:513, :] = xc
        v = xp.reshape(IMG, H, NG, G, C)                  # [img, r, g, wi, ci]
        xdc = v.transpose(3, 4, 0, 1, 2)                  # [wi, ci, img, r, g]
        xdp = np.zeros((G, C, IMG, HP, NG), ml_dtypes.bfloat16)
        xdp[:, :, :, 1:H + 1, :] = xdc
        in_maps.append({
            "xd": np.ascontiguousarray(xdp).reshape(-1),
            "wa": wa_d, "wt": wt_d, "bias": bias128,
        })
    return in_maps


def _assemble(results) -> np.ndarray:
    # dev [128=(j,co), img, r, g] per core -> NHWC [16, 512, 512, 16] f32
    dev = np.stack([r["out"].reshape(G, C, IMG, H, NGO) for r in results])
    out = dev.transpose(0, 3, 4, 5, 1, 2).reshape(16, H, W, C)
    return np.ascontiguousarray(out.astype(np.float32))


def kernel(x: np.ndarray, filters: np.ndarray, bias: np.ndarray) -> np.ndarray:
    in_maps = _prep_inputs(x, filters, bias)
    nc = _get_nc()
    res = run_bass_kernel_spmd(nc, in_maps, core_ids=list(range(N_CORES)))
    return _assemble(res.results)


# revision 3
# speedup vs baseline: 8.6403x; 1.0036x over previous
"""Trainium2 Bass kernel for 3x3 SAME conv (NHWC, 16x512x512x16, C=16) + bias.

Strategy (8 NeuronCores, data-parallel over batch; 2 images per core):
  - Host pre-arranges x (bf16) into two partition-blocked tensors:
      xc[(wi,ci)=128][img, rpad, g=0..63]  holding x[r, 8g-1+wi, ci]
      xt[(k, ci)=32 ][img, rpad, g=0..63]  holding x[r, 8g+7+k, ci]
    (SAME padding baked in as zeros).  Every device DMA is plain and
    per-partition contiguous, and every matmul rhs is one contiguous
    512-element run - no transpose DMAs, no strided access patterns.
  - Conv per 8-row PSUM bank = 6 accumulating bf16 matmuls:
      3x center (K=128): Wa[dy]^T @ XC[:, t+dy : t+dy+8, :]
      3x tail   (K=32):  Wt[dy]^T @ XT[:, t+dy : t+dy+8, :]
  - DVE adds bias during PSUM->SBUF copy, casting to bf16; one store DMA
    per chunk writes partition-blocked contiguous runs; host reorders to
    NHWC f32.
"""

from contextlib import ExitStack

import ml_dtypes
import numpy as np

import concourse.bass as bass
import concourse.bacc as bacc
import concourse.mybir as mybir
import concourse.tile as tile
from concourse.bass_utils import run_bass_kernel_spmd

F32 = mybir.dt.float32
BF16 = mybir.dt.bfloat16

N_CORES = 8
H = 512
W = 512
C = 16
IMG = 2                  # images per core
G = 8                    # output pixels per group
NGO = 64                 # groups per row
RC = 64                  # output rows per chunk
NCHUNK = H // RC         # 8 chunks per image
TIN = RC + 2             # input rows per chunk (halo)
RB = 8                   # output rows per PSUM bank
NBANK = RC // RB         # 8 banks per chunk
HP = H + 2               # padded rows per image
PBLK = IMG * HP * NGO    # per-partition elements in xc/xt
OBLK = IMG * H * NGO     # per-partition elements in out


def _build_nc():
    nc = bacc.Bacc(None, target_bir_lowering=False)
    xc = nc.dram_tensor("xc", [128 * PBLK], BF16, kind="ExternalInput")
    xt = nc.dram_tensor("xt", [32 * PBLK], BF16, kind="ExternalInput")
    wa = nc.dram_tensor("wa", [128, 3, 128], BF16, kind="ExternalInput")
    wt = nc.dram_tensor("wt", [32, 3, 128], BF16, kind="ExternalInput")
    bias = nc.dram_tensor("bias", [128, 1], F32, kind="ExternalInput")
    out = nc.dram_tensor("out", [128 * OBLK], BF16, kind="ExternalOutput")

    with ExitStack() as ctx:
        tc = ctx.enter_context(tile.TileContext(nc))
        wpool = ctx.enter_context(tc.tile_pool(name="w", bufs=1))
        xpool = ctx.enter_context(tc.tile_pool(name="x", bufs=3))
        opool = ctx.enter_context(tc.tile_pool(name="o", bufs=2))
        pspool = ctx.enter_context(tc.tile_pool(name="ps", bufs=8, space="PSUM"))

        wat = wpool.tile([128, 3, 128], BF16)
        nc.sync.dma_start(wat[:, :, :], wa[:, :, :])
        wtt = wpool.tile([32, 3, 128], BF16)
        nc.sync.dma_start(wtt[:, :, :], wt[:, :, :])
        bias_t = wpool.tile([128, 1], F32)
        nc.sync.dma_start(bias_t[:, :], bias[:, :])

        for img in range(IMG):
            for ck in range(NCHUNK):
                r0 = ck * RC
                off = (img * HP + r0) * NGO

                XC = xpool.tile([128, TIN, NGO], BF16, tag="xc")
                nc.sync.dma_start(
                    XC[:, :, :].rearrange("p t g -> p (t g)"),
                    bass.AP(xc, off, [[PBLK, 128], [1, TIN * NGO]]),
                )
                XT = xpool.tile([32, TIN, NGO], BF16, tag="xt")
                nc.sync.dma_start(
                    XT[:, :, :].rearrange("p t g -> p (t g)"),
                    bass.AP(xt, off, [[PBLK, 32], [1, TIN * NGO]]),
                )

                O = opool.tile([128, RC, NGO], BF16, tag="o")
                for b in range(NBANK):
                    t0 = RB * b
                    ps = pspool.tile([128, RB, NGO], F32, tag="ps")
                    for dy in range(3):
                        nc.tensor.matmul(
                            ps[:, :, :], wat[:, dy, :],
                            XC[:, t0 + dy:t0 + dy + RB, :],
                            start=(dy == 0), stop=False)
                    for dy in range(3):
                        nc.tensor.matmul(
                            ps[:, :, :], wtt[:, dy, :],
                            XT[:, t0 + dy:t0 + dy + RB, :],
                            start=False, stop=(dy == 2))
                    nc.vector.tensor_scalar_add(
                        out=O[:, t0:t0 + RB, :], in0=ps[:, :, :],
                        scalar1=bias_t[:, 0:1])

                nc.sync.dma_start(
                    bass.AP(out, (img * H + r0) * NGO,
                            [[OBLK, 128], [1, RC * NGO]]),
                    O[:, :, :].rearrange("p t g -> p (t g)"),
                )
    nc.finalize()
    return nc


_NC_CACHE = None


def _get_nc():
    global _NC_CACHE
    if _NC_CACHE is None:
        _NC_CACHE = _build_nc()
    return _NC_CACHE


def _weights(filters: np.ndarray):
    """filters (3,3,16,16) HWIO -> wa [128,3,128], wt [32,3,128] bf16."""
    wa = np.zeros((3, 8, 16, 8, 16), np.float32)   # [dy, wi, ci, j, co]
    wt = np.zeros((3, 2, 16, 8, 16), np.float32)   # [dy, k, ci, j, co]
    for dy in range(3):
        for dx in range(3):
            for j in range(G):
                wi = j + dx
                if wi <= 7:
                    wa[dy, wi, :, j, :] = filters[dy, dx]
                else:
                    wt[dy, wi - 8, :, j, :] = filters[dy, dx]
    wa_d = np.ascontiguousarray(
        wa.reshape(3, 128, 128).transpose(1, 0, 2)).astype(ml_dtypes.bfloat16)
    wt_d = np.ascontiguousarray(
        wt.reshape(3, 32, 128).transpose(1, 0, 2)).astype(ml_dtypes.bfloat16)
    return wa_d, wt_d


def _prep_inputs(x, filters, bias):
    x = np.asarray(x, dtype=np.float32)
    filters = np.asarray(filters, dtype=np.float32)
    bias = np.asarray(bias, dtype=np.float32)
    assert x.shape == (16, H, W, C), x.shape

    wa_d, wt_d = _weights(filters)
    bias128 = np.ascontiguousarray(
        np.tile(bias, G).reshape(128, 1)).astype(np.float32)

    x_bf = x.astype(ml_dtypes.bfloat16)
    in_maps = []
    for i in range(N_CORES):
        xi = x_bf[i * IMG:(i + 1) * IMG]                  # [2, 512, 512, 16]
        # w-padded: idx 0..519 maps to w = idx-1 (zeros outside [0,512))
        xp = np.zeros((IMG, H, 520, C), ml_dtypes.bfloat16)
        xp[:, :, 1:513, :] = xi
        # center: w = 8g-1+wi  <-> idx = 8g+wi, wi=0..7
        vc = xp[:, :, 0:512, :].reshape(IMG, H, NGO, G, C)
        xcc = vc.transpose(3, 4, 0, 1, 2).reshape(128, IMG, H, NGO)
        # tail: w = 8g+7+k <-> idx = 8g+8+k, k=0..1
        vt = xp[:, :, 8:520, :].reshape(IMG, H, NGO, G, C)[:, :, :, 0:2, :]
        xtc = vt.transpose(3, 4, 0, 1, 2).reshape(32, IMG, H, NGO)
        # row padding (1 zero row before/after each image)
        xcp = np.zeros((128, IMG, HP, NGO), ml_dtypes.bfloat16)
        xcp[:, :, 1:H + 1, :] = xcc
        xtp = np.zeros((32, IMG, HP, NGO), ml_dtypes.bfloat16)
        xtp[:, :, 1:H + 1, :] = xtc
        in_maps.append({
            "xc": np.ascontiguousarray(xcp).reshape(-1),
            "xt": np.ascontiguousarray(xtp).reshape(-1),
            "wa": wa_d, "wt": wt_d, "bias": bias128,
        })
    return in_maps


def _assemble(results) -> np.ndarray:
    # dev [128=(j,co), img, r, g] per core -> NHWC [16, 512, 512, 16] f32
    dev = np.stack([r["out"].reshape(G, C, IMG, H, NGO) for r in results])
    out = dev.transpose(0, 3, 4, 5, 1, 2).reshape(16, H, W, C)
    return np.ascontiguousarray(out.astype(np.float32))


def kernel(x: np.ndarray, filters: np.ndarray, bias: np.ndarray) -> np.ndarray:
    in_maps = _prep_inputs(x, filters, bias)
    nc = _get_nc()
    res = run_bass_kernel_spmd(nc, in_maps, core_ids=list(range(N_CORES)))
    return _assemble(res.results)


# revision 16
# speedup vs baseline: 18.3728x; 2.1264x over previous
"""Trainium2 Bass kernel for 3x3 SAME conv (NHWC, 16x512x512x16, C=16) + bias.

Strategy (8 NeuronCores, data-parallel over batch; 2 images per core):
  - Host pre-arranges x (bf16) into two partition-blocked tensors:
      xc[(wi,ci)=128][img, rpad, g=0..63]  holding x[r, 8g-1+wi, ci]
      xt[(k, ci)=32 ][img, rpad, g=0..63]  holding x[r, 8g+7+k, ci]
    (SAME padding baked in as zeros).  Every device DMA is plain and
    per-partition contiguous, and every matmul rhs is one contiguous
    512-element run - no transpose DMAs, no strided access patterns.
  - Conv per 8-row PSUM bank = 4 accumulating bf16 matmuls:
      3x center (K=128): Wa[dy]^T @ XC[:, t+dy : t+dy+8, :]
      1x tail   (K=96):  Wt3^T   @ XT3[:, t : t+8, :]
    The tail tensor holds three 32-partition strips, strip dy PRE-SHIFTED
    by dy rows on the host, so the three per-dy K=32 tail contractions
    collapse into one K=96 matmul - 4 matmul slots per bank instead of 6.
  - DVE adds bias during PSUM->SBUF copy, casting to bf16; one store DMA
    per chunk writes partition-blocked contiguous runs; host reorders to
    NHWC f32.
"""

from contextlib import ExitStack

import ml_dtypes
import numpy as np

import concourse.bass as bass
import concourse.bacc as bacc
import concourse.mybir as mybir
import concourse.tile as tile
from concourse.bass_utils import run_bass_kernel_spmd

F32 = mybir.dt.float32
BF16 = mybir.dt.bfloat16

N_CORES = 8
H = 512
W = 512
C = 16
IMG = 2                  # images per core
G = 8                    # output pixels per group
NGO = 64                 # groups per row
RC = 64                  # output rows per chunk
NCHUNK = H // RC         # 8 chunks per image
TIN = RC + 2             # input rows per chunk (halo)
RB = 8                   # output rows per PSUM bank
NBANK = RC // RB         # 8 banks per chunk
HP = H + 2               # padded rows per image (center)
HPS = H + 4              # padded rows per image (tail, +2 for dy shift)
PBLK = IMG * HP * NGO    # per-partition elements in xc
PBLKT = IMG * HPS * NGO  # per-partition elements in xt
OBLK = IMG * H * NGO     # per-partition elements in out


def _build_nc():
    nc = bacc.Bacc(None, target_bir_lowering=False)
    xc = nc.dram_tensor("xc", [128 * PBLK], BF16, kind="ExternalInput")
    xt = nc.dram_tensor("xt", [96 * PBLKT], BF16, kind="ExternalInput")
    wa = nc.dram_tensor("wa", [128, 3, 128], BF16, kind="ExternalInput")
    wt = nc.dram_tensor("wt", [96, 128], BF16, kind="ExternalInput")
    bias = nc.dram_tensor("bias", [128, 1], F32, kind="ExternalInput")
    out = nc.dram_tensor("out", [128 * OBLK], BF16, kind="ExternalOutput")

    with ExitStack() as ctx:
        tc = ctx.enter_context(tile.TileContext(nc))
        wpool = ctx.enter_context(tc.tile_pool(name="w", bufs=1))
        xpool = ctx.enter_context(tc.tile_pool(name="x", bufs=3))
        opool = ctx.enter_context(tc.tile_pool(name="o", bufs=2))
        pspool = ctx.enter_context(tc.tile_pool(name="ps", bufs=8, space="PSUM"))

        wat = wpool.tile([128, 3, 128], BF16)
        nc.sync.dma_start(wat[:, :, :], wa[:, :, :])
        wtt = wpool.tile([96, 128], BF16)
        nc.sync.dma_start(wtt[:, :], wt[:, :])
        bias_t = wpool.tile([128, 1], F32)
        nc.sync.dma_start(bias_t[:, :], bias[:, :])

        for img in range(IMG):
            for ck in range(NCHUNK):
                r0 = ck * RC
                off = (img * HP + r0) * NGO

                XC = xpool.tile([128, TIN, NGO], BF16, tag="xc")
                nc.sync.dma_start(
                    XC[:, :, :].rearrange("p t g -> p (t g)"),
                    bass.AP(xc, off, [[PBLK, 128], [1, TIN * NGO]]),
                )
                XT = xpool.tile([96, RC, NGO], BF16, tag="xt")
                nc.sync.dma_start(
                    XT[:, :, :].rearrange("p t g -> p (t g)"),
                    bass.AP(xt, (img * HPS + r0) * NGO,
                            [[PBLKT, 96], [1, RC * NGO]]),
                )

                O = opool.tile([128, RC, NGO], BF16, tag="o")
                for b in range(NBANK):
                    t0 = RB * b
                    ps = pspool.tile([128, RB, NGO], F32, tag="ps")
                    for dy in range(3):
                        nc.tensor.matmul(
                            ps[:, :, :], wat[:, dy, :],
                            XC[:, t0 + dy:t0 + dy + RB, :],
                            start=(dy == 0), stop=False)
                    nc.tensor.matmul(
                        ps[:, :, :], wtt[:, :],
                        XT[:, t0:t0 + RB, :],
                        start=False, stop=True)
                    nc.vector.tensor_scalar_add(
                        out=O[:, t0:t0 + RB, :], in0=ps[:, :, :],
                        scalar1=bias_t[:, 0:1])

                nc.sync.dma_start(
                    bass.AP(out, (img * H + r0) * NGO,
                            [[OBLK, 128], [1, RC * NGO]]),
                    O[:, :, :].rearrange("p t g -> p (t g)"),
                )
    nc.finalize()
    return nc


_NC_CACHE = None


def _get_nc():
    global _NC_CACHE
    if _NC_CACHE is None:
        _NC_CACHE = _build_nc()
    return _NC_CACHE


def _weights(filters: np.ndarray):
    """filters (3,3,16,16) HWIO -> wa [128,3,128], wt [32,3,128] bf16."""
    wa = np.zeros((3, 8, 16, 8, 16), np.float32)   # [dy, wi, ci, j, co]
    wt = np.zeros((3, 2, 16, 8, 16), np.float32)   # [dy, k, ci, j, co]
    for dy in range(3):
        for dx in range(3):
            for j in range(G):
                wi = j + dx
                if wi <= 7:
                    wa[dy, wi, :, j, :] = filters[dy, dx]
                else:
                    wt[dy, wi - 8, :, j, :] = filters[dy, dx]
    wa_d = np.ascontiguousarray(
        wa.reshape(3, 128, 128).transpose(1, 0, 2)).astype(ml_dtypes.bfloat16)
    # wt3 [96, 128]: rows 32dy..32dy+32 hold Wt[dy] (strip dy of the PE array)
    wt_d = np.ascontiguousarray(
        wt.reshape(3, 32, 128).reshape(96, 128)).astype(ml_dtypes.bfloat16)
    return wa_d, wt_d


def _prep_inputs(x, filters, bias):
    x = np.asarray(x, dtype=np.float32)
    filters = np.asarray(filters, dtype=np.float32)
    bias = np.asarray(bias, dtype=np.float32)
    assert x.shape == (16, H, W, C), x.shape

    wa_d, wt_d = _weights(filters)
    bias128 = np.ascontiguousarray(
        np.tile(bias, G).reshape(128, 1)).astype(np.float32)

    x_bf = x.astype(ml_dtypes.bfloat16)
    in_maps = []
    for i in range(N_CORES):
        xi = x_bf[i * IMG:(i + 1) * IMG]                  # [2, 512, 512, 16]
        # w-padded: idx 0..519 maps to w = idx-1 (zeros outside [0,512))
        xp = np.zeros((IMG, H, 520, C), ml_dtypes.bfloat16)
        xp[:, :, 1:513, :] = xi
        # center: w = 8g-1+wi  <-> idx = 8g+wi, wi=0..7
        vc = xp[:, :, 0:512, :].reshape(IMG, H, NGO, G, C)
        xcc = vc.transpose(3, 4, 0, 1, 2).reshape(128, IMG, H, NGO)
        # tail: w = 8g+7+k <-> idx = 8g+8+k, k=0..1
        vt = xp[:, :, 8:520, :].reshape(IMG, H, NGO, G, C)[:, :, :, 0:2, :]
        xtc = vt.transpose(3, 4, 0, 1, 2).reshape(32, IMG, H, NGO)
        # row padding (1 zero row before/after each image)
        xcp = np.zeros((128, IMG, HP, NGO), ml_dtypes.bfloat16)
        xcp[:, :, 1:H + 1, :] = xcc
        xtp = np.zeros((32, IMG, HPS, NGO), ml_dtypes.bfloat16)
        xtp[:, :, 1:H + 1, :] = xtc
        # strip dy pre-shifted by dy rows so one K=96 matmul covers all dy
        strips = []
        for dy in range(3):
            s = np.zeros_like(xtp)
            s[:, :, 0:HPS - dy, :] = xtp[:, :, dy:HPS, :]
            strips.append(s)
        xt3 = np.concatenate(strips, 0)        # [96, IMG, HPS, NGO]
        in_maps.append({
            "xc": np.ascontiguousarray(xcp).reshape(-1),
            "xt": np.ascontiguousarray(xt3).reshape(-1),
            "wa": wa_d, "wt": wt_d, "bias": bias128,
        })
    return in_maps


def _assemble(results) -> np.ndarray:
    # dev [128=(j,co), img, r, g] per core -> NHWC [16, 512, 512, 16] f32
    dev = np.stack([r["out"].reshape(G, C, IMG, H, NGO) for r in results])
    out = dev.transpose(0, 3, 4, 5, 1, 2).reshape(16, H, W, C)
    return np.ascontiguousarray(out.astype(np.float32))


def kernel(x: np.ndarray, filters: np.ndarray, bias: np.ndarray) -> np.ndarray:
    in_maps = _prep_inputs(x, filters, bias)
    nc = _get_nc()
    res = run_bass_kernel_spmd(nc, in_maps, core_ids=list(range(N_CORES)))
    return _assemble(res.results)


# revision 18
# speedup vs baseline: 18.7827x; 1.0223x over previous
"""Trainium2 Bass kernel for 3x3 SAME conv (NHWC, 16x512x512x16, C=16) + bias.

Strategy (8 NeuronCores, data-parallel over batch; 2 images per core):
  - Host pre-arranges x (bf16) into two partition-blocked tensors:
      xc[(wi,ci)=128][img, rpad, g=0..63]  holding x[r, 8g-1+wi, ci]
      xt[(k, ci)=32 ][img, rpad, g=0..63]  holding x[r, 8g+7+k, ci]
    (SAME padding baked in as zeros).  Every device DMA is plain and
    per-partition contiguous, and every matmul rhs is one contiguous
    512-element run - no transpose DMAs, no strided access patterns.
  - Conv per 8-row PSUM bank = 4 accumulating bf16 matmuls:
      3x center (K=128): Wa[dy]^T @ XC[:, t+dy : t+dy+8, :]
      1x tail   (K=96):  Wt3^T   @ XT3[:, t : t+8, :]
    The tail tensor holds three 32-partition strips, strip dy PRE-SHIFTED
    by dy rows on the host, so the three per-dy K=32 tail contractions
    collapse into one K=96 matmul - 4 matmul slots per bank instead of 6.
  - DVE adds bias during PSUM->SBUF copy, casting to bf16; one store DMA
    per chunk writes partition-blocked contiguous runs; host reorders to
    NHWC f32.
"""

from contextlib import ExitStack

import ml_dtypes
import numpy as np

import concourse.bass as bass
import concourse.bacc as bacc
import concourse.mybir as mybir
import concourse.tile as tile
from concourse.bass_utils import run_bass_kernel_spmd

F32 = mybir.dt.float32
BF16 = mybir.dt.bfloat16

N_CORES = 8
H = 512
W = 512
C = 16
IMG = 2                  # images per core
G = 8                    # output pixels per group
NGO = 64                 # groups per row
RC = 64                  # output rows per chunk
NCHUNK = H // RC         # 8 chunks per image
TIN = RC + 2             # input rows per chunk (halo)
RB = 8                   # output rows per PSUM bank
NBANK = RC // RB         # 8 banks per chunk
HP = H + 2               # padded rows per image (center)
HPS = H + 4              # padded rows per image (tail, +2 for dy shift)
PBLK = IMG * HP * NGO    # per-partition elements in xc
PBLKT = IMG * HPS * NGO  # per-partition elements in xt
OBLK = IMG * H * NGO     # per-partition elements in out


def _build_nc():
    nc = bacc.Bacc(None, target_bir_lowering=False)
    xc = nc.dram_tensor("xc", [128 * PBLK], BF16, kind="ExternalInput")
    xt = nc.dram_tensor("xt", [96 * PBLKT], BF16, kind="ExternalInput")
    wa = nc.dram_tensor("wa", [128, 3, 128], BF16, kind="ExternalInput")
    wt = nc.dram_tensor("wt", [96, 128], BF16, kind="ExternalInput")
    bias = nc.dram_tensor("bias", [128, 1], F32, kind="ExternalInput")
    out = nc.dram_tensor("out", [128 * OBLK], BF16, kind="ExternalOutput")

    with ExitStack() as ctx:
        tc = ctx.enter_context(tile.TileContext(nc))
        wpool = ctx.enter_context(tc.tile_pool(name="w", bufs=1))
        xpool = ctx.enter_context(tc.tile_pool(name="x", bufs=4))
        opool = ctx.enter_context(tc.tile_pool(name="o", bufs=3))
        pspool = ctx.enter_context(tc.tile_pool(name="ps", bufs=8, space="PSUM"))

        wat = wpool.tile([128, 3, 128], BF16)
        nc.sync.dma_start(wat[:, :, :], wa[:, :, :])
        wtt = wpool.tile([96, 128], BF16)
        nc.sync.dma_start(wtt[:, :], wt[:, :])
        bias_t = wpool.tile([128, 1], F32)
        nc.sync.dma_start(bias_t[:, :], bias[:, :])

        for img in range(IMG):
            for ck in range(NCHUNK):
                r0 = ck * RC
                off = (img * HP + r0) * NGO

                XC = xpool.tile([128, TIN, NGO], BF16, tag="xc")
                nc.sync.dma_start(
                    XC[:, :, :].rearrange("p t g -> p (t g)"),
                    bass.AP(xc, off, [[PBLK, 128], [1, TIN * NGO]]),
                )
                XT = xpool.tile([96, RC, NGO], BF16, tag="xt")
                nc.sync.dma_start(
                    XT[:, :, :].rearrange("p t g -> p (t g)"),
                    bass.AP(xt, (img * HPS + r0) * NGO,
                            [[PBLKT, 96], [1, RC * NGO]]),
                )

                O = opool.tile([128, RC, NGO], BF16, tag="o")
                for b in range(NBANK):
                    t0 = RB * b
                    ps = pspool.tile([128, RB, NGO], F32, tag="ps")
                    for dy in range(3):
                        nc.tensor.matmul(
                            ps[:, :, :], wat[:, dy, :],
                            XC[:, t0 + dy:t0 + dy + RB, :],
                            start=(dy == 0), stop=False)
                    nc.tensor.matmul(
                        ps[:, :, :], wtt[:, :],
                        XT[:, t0:t0 + RB, :],
                        start=False, stop=True)
                    # PSUM drain + bias: alternate DVE / ACT so neither
                    # engine serializes the per-chunk pipeline
                    if b % 2 == 0:
                        nc.vector.tensor_scalar_add(
                            out=O[:, t0:t0 + RB, :], in0=ps[:, :, :],
                            scalar1=bias_t[:, 0:1])
                    else:
                        nc.scalar.activation(
                            O[:, t0:t0 + RB, :], ps[:, :, :],
                            mybir.ActivationFunctionType.Identity,
                            bias=bias_t[:, 0:1], scale=1.0)

                nc.sync.dma_start(
                    bass.AP(out, (img * H + r0) * NGO,
                            [[OBLK, 128], [1, RC * NGO]]),
                    O[:, :, :].rearrange("p t g -> p (t g)"),
                )
    nc.finalize()
    return nc


_NC_CACHE = None


def _get_nc():
    global _NC_CACHE
    if _NC_CACHE is None:
        _NC_CACHE = _build_nc()
    return _NC_CACHE


def _weights(filters: np.ndarray):
    """filters (3,3,16,16) HWIO -> wa [128,3,128], wt [32,3,128] bf16."""
    wa = np.zeros((3, 8, 16, 8, 16), np.float32)   # [dy, wi, ci, j, co]
    wt = np.zeros((3, 2, 16, 8, 16), np.float32)   # [dy, k, ci, j, co]
    for dy in range(3):
        for dx in range(3):
            for j in range(G):
                wi = j + dx
                if wi <= 7:
                    wa[dy, wi, :, j, :] = filters[dy, dx]
                else:
                    wt[dy, wi - 8, :, j, :] = filters[dy, dx]
    wa_d = np.ascontiguousarray(
        wa.reshape(3, 128, 128).transpose(1, 0, 2)).astype(ml_dtypes.bfloat16)
    # wt3 [96, 128]: rows 32dy..32dy+32 hold Wt[dy] (strip dy of the PE array)
    wt_d = np.ascontiguousarray(
        wt.reshape(3, 32, 128).reshape(96, 128)).astype(ml_dtypes.bfloat16)
    return wa_d, wt_d


def _prep_inputs(x, filters, bias):
    x = np.asarray(x, dtype=np.float32)
    filters = np.asarray(filters, dtype=np.float32)
    bias = np.asarray(bias, dtype=np.float32)
    assert x.shape == (16, H, W, C), x.shape

    wa_d, wt_d = _weights(filters)
    bias128 = np.ascontiguousarray(
        np.tile(bias, G).reshape(128, 1)).astype(np.float32)

    x_bf = x.astype(ml_dtypes.bfloat16)
    in_maps = []
    for i in range(N_CORES):
        xi = x_bf[i * IMG:(i + 1) * IMG]                  # [2, 512, 512, 16]
        # w-padded: idx 0..519 maps to w = idx-1 (zeros outside [0,512))
        xp = np.zeros((IMG, H, 520, C), ml_dtypes.bfloat16)
        xp[:, :, 1:513, :] = xi
        # center: w = 8g-1+wi  <-> idx = 8g+wi, wi=0..7
        vc = xp[:, :, 0:512, :].reshape(IMG, H, NGO, G, C)
        xcc = vc.transpose(3, 4, 0, 1, 2).reshape(128, IMG, H, NGO)
        # tail: w = 8g+7+k <-> idx = 8g+8+k, k=0..1
        vt = xp[:, :, 8:520, :].reshape(IMG, H, NGO, G, C)[:, :, :, 0:2, :]
        xtc = vt.transpose(3, 4, 0, 1, 2).reshape(32, IMG, H, NGO)
        # row padding (1 zero row before/after each image)
        xcp = np.zeros((128, IMG, HP, NGO), ml_dtypes.bfloat16)
        xcp[:, :, 1:H + 1, :] = xcc
        xtp = np.zeros((32, IMG, HPS, NGO), ml_dtypes.bfloat16)
        xtp[:, :, 1:H + 1, :] = xtc
        # strip dy pre-shifted by dy rows so one K=96 matmul covers all dy
        strips = []
        for dy in range(3):
            s = np.zeros_like(xtp)
            s[:, :, 0:HPS - dy, :] = xtp[:, :, dy:HPS, :]
            strips.append(s)
        xt3 = np.concatenate(strips, 0)        # [96, IMG, HPS, NGO]
        in_maps.append({
            "xc": np.ascontiguousarray(xcp).reshape(-1),
            "xt": np.ascontiguousarray(xt3).reshape(-1),
            "wa": wa_d, "wt": wt_d, "bias": bias128,
        })
    return in_maps


def _assemble(results) -> np.ndarray:
    # dev [128=(j,co), img, r, g] per core -> NHWC [16, 512, 512, 16] f32
    dev = np.stack([r["out"].reshape(G, C, IMG, H, NGO) for r in results])
    out = dev.transpose(0, 3, 4, 5, 1, 2).reshape(16, H, W, C)
    return np.ascontiguousarray(out.astype(np.float32))


def kernel(x: np.ndarray, filters: np.ndarray, bias: np.ndarray) -> np.ndarray:
    in_maps = _prep_inputs(x, filters, bias)
    nc = _get_nc()
    res = run_bass_kernel_spmd(nc, in_maps, core_ids=list(range(N_CORES)))
    return _assemble(res.results)
